# revision 4
# baseline (speedup 1.0000x reference)
"""LightGCN 3-layer propagation + CVIB loss on 8 Trainium2 NeuronCores.

Self-contained kernel: takes full (unsharded) numpy inputs, row-shards the
graph across 8 cores, runs a Bass/Tile SPMD program (gather + one-hot-matmul
segment-sum SpMM per layer, AllGather between layers, data-parallel loss),
and combines per-core partial sums on the host into the two scalar losses.
"""
import sys

sys.path.insert(0, "/opt/trn_rl_repo")

import numpy as np
import ml_dtypes

import concourse.bass as bass
import concourse.bacc as bacc
import concourse.tile as tile
from concourse import mybir
from concourse.bass_utils import run_bass_kernel_spmd

# ---------------- problem constants ----------------
N_USERS = 100000
N_ITEMS = 40000
N_NODES = N_USERS + N_ITEMS
EMB = 128
BATCH = 8192
N_LAYERS = 3
ALPHA = 0.1
GAMMA = 0.01

# ---------------- sharding config ----------------
P = 128
NCORES = 8
TILES = 137                      # row tiles per core
RPC = TILES * P                  # rows per core = 17536
NT = NCORES * RPC                # padded table rows = 140288
CHUNK = 32768                    # int16 gather index range
NCHUNK = (NT + CHUNK - 1) // CHUNK   # 5

# tunables
SB_T = 5                         # tiles per gather superblock
TABLE_BF16 = True                # table / gather / matmul dtype
DEBUG_SB_LIMIT = None            # process only first N superblocks (debug)
DEBUG_SKIP_LOSS = False
DEBUG_LAYERS = N_LAYERS
GMAX = 1024                      # max idxs per dma_gather (HW ring limit)
DEBUG_SINGLE = False             # 1-core, collectives replaced by local copies
DEBUG_LOCAL_COLL = False         # 8-core, collectives replaced by local copies

DT = mybir.dt.bfloat16 if TABLE_BF16 else mybir.dt.float32
NPDT = ml_dtypes.bfloat16 if TABLE_BF16 else np.float32


def cdiv(a, b):
    return (a + b - 1) // b


# ============================================================
# Host-side packing
# ============================================================

def _wrap_idx(lst):
    """int16 list (len % 16 == 0) -> wrapped [16, len/16] block."""
    return lst.reshape(-1, 16).T


class Plan:
    """Static (data-shape) schedule shared by all cores."""
    pass


def host_pack(user_emb, item_emb, graph_vals, edge_user, edge_item,
              users, pos_items, neg_items, sampled_user, sampled_items):
    rows0 = np.concatenate([edge_user, edge_item + N_USERS]).astype(np.int64)
    cols0 = np.concatenate([edge_item + N_USERS, edge_user]).astype(np.int64)
    vals = np.asarray(graph_vals, np.float32)

    # node -> table-row permutation: snake-deal nodes by degree across cores so
    # every core gets a near-identical degree profile per tile index, and cols
    # spread evenly over int16 gather chunks.
    deg = np.bincount(rows0, minlength=N_NODES)
    order = np.argsort(-deg, kind="stable")
    perm = np.empty(N_NODES, np.int64)
    k = np.arange(N_NODES)
    perm[order] = (k % NCORES) * RPC + k // NCORES
    rows = perm[rows0]
    cols = perm[cols0]

    core_of = rows // RPC
    tile_of = (rows % RPC) // P
    lrow = (rows % P).astype(np.float32)
    chunk = cols // CHUNK
    lcol = (cols % CHUNK).astype(np.int16)

    # group edges by (core, tile, chunk)
    key = (core_of * TILES + tile_of) * NCHUNK + chunk
    order = np.argsort(key, kind="stable")
    key_s = key[order]
    lcol_s = lcol[order]
    lrow_s = lrow[order]
    vals_s = vals[order]
    ncell = NCORES * TILES * NCHUNK
    counts = np.bincount(key_s, minlength=ncell).reshape(NCORES, TILES, NCHUNK)
    starts = np.zeros(ncell + 1, np.int64)
    np.cumsum(counts.reshape(-1), out=starts[1:])

    # static slot budgets: max over cores
    B = cdiv(counts, P).max(axis=0)          # [TILES, NCHUNK] slots
    n_sb = cdiv(TILES, SB_T)
    sb_tiles = [list(range(s * SB_T, min((s + 1) * SB_T, TILES))) for s in range(n_sb)]

    plan = Plan()
    plan.B = B
    plan.sb_tiles = sb_tiles
    # slot offset of (tile, chunk) region within the GLOBAL slot stream, plus
    # per-sb slot extents. Layout per sb: chunk-major [c][t].
    slot_off = np.zeros((TILES, NCHUNK), np.int64)
    gathers = []   # (sb, c, idx_col_off, n_idx, dst_slot_off_in_sb, sb_index)
    sb_slot_base = []   # global slot base of each sb
    sb_nslots = []
    g_off = 0          # global slot counter
    idx_cols = 0       # idx array column counter (int16 cols of 16)
    for s, tl in enumerate(sb_tiles):
        sb_slot_base.append(g_off)
        sb_start = g_off
        for c in range(NCHUNK):
            n_slots = int(B[tl, c].sum())
            if n_slots == 0:
                continue
            n_idx = n_slots * P
            gathers.append(dict(sb=s, c=c, idx_col=idx_cols, n_idx=n_idx,
                                dst_slot=g_off - sb_start))
            # per-tile offsets inside this (sb, c) region
            off = g_off
            for t in tl:
                slot_off[t, c] = off
                off += int(B[t, c])
            idx_cols += n_idx // 16
            g_off += n_slots
        sb_nslots.append(g_off - sb_start)
    plan.slot_off = slot_off
    plan.gathers = gathers
    plan.sb_slot_base = sb_slot_base
    plan.sb_nslots = sb_nslots
    plan.tot_slots = g_off
    plan.g_slots = max(sb_nslots)
    plan.idx_cols_edges = idx_cols

    # ---- batch (loss) packing: pair sets sorted by (chunk_a, chunk_b) ----
    users = perm[np.asarray(users, np.int64)]
    pos_t = perm[np.asarray(pos_items, np.int64) + N_USERS]
    neg_t = perm[np.asarray(neg_items, np.int64) + N_USERS]
    su = perm[np.asarray(sampled_user, np.int64)]
    si = perm[np.asarray(sampled_items, np.int64) + N_USERS]

    bpc = BATCH // NCORES          # 1024 per core
    spc = 2 * BATCH // NCORES      # 2048 per core

    def pack_pairs(a_all, b_all, n_per_core):
        """Pair set: per core slice; sort pairs by (chunk_a, chunk_b); pad each
        (ca, cb) cell to multiple of P with dup pads (mask 0).
        Returns static cell budgets + per-core data."""
        a_all = a_all.reshape(NCORES, n_per_core)
        b_all = b_all.reshape(NCORES, n_per_core)
        ca = a_all // CHUNK
        cb = b_all // CHUNK
        cell = ca * NCHUNK + cb
        counts = np.stack([np.bincount(cell[i], minlength=NCHUNK * NCHUNK)
                           for i in range(NCORES)])     # [NCORES, 25]
        cellB = cdiv(counts, P).max(axis=0)             # slots per cell (static)
        per_core = []
        for i in range(NCORES):
            o = np.argsort(cell[i], kind="stable")
            a_s, b_s, cell_s = a_all[i][o], b_all[i][o], cell[i][o]
            a_out, b_out, m_out = [], [], []
            st = 0
            for cc in range(NCHUNK * NCHUNK):
                n = int(counts[i, cc])
                want = int(cellB[cc]) * P
                if want == 0:
                    assert n == 0
                    continue
                a_c = a_s[st:st + n]
                b_c = b_s[st:st + n]
                st += n
                pa = np.full(want - n, (a_c[0] if n else (cc // NCHUNK) * CHUNK),
                             np.int64)
                pb = np.full(want - n, (b_c[0] if n else (cc % NCHUNK) * CHUNK),
                             np.int64)
                a_out.append(np.concatenate([a_c, pa]))
                b_out.append(np.concatenate([b_c, pb]))
                m = np.zeros(want, np.float32)
                m[:n] = 1.0
                m_out.append(m)
            per_core.append((np.concatenate(a_out), np.concatenate(b_out),
                             np.concatenate(m_out)))
        return cellB, per_core

    plan.setA = pack_pairs(users, pos_t, bpc)       # (u, pos)
    plan.setB = pack_pairs(users, neg_t, bpc)       # (u, neg)
    plan.setC = pack_pairs(su, si, spc)             # (su, si)

    def set_slots(cellB):
        return int(cellB.sum())

    plan.sA = set_slots(plan.setA[0])
    plan.sB = set_slots(plan.setB[0])
    plan.sC = set_slots(plan.setC[0])

    # gather call list for a pair set: for the "a" side one gather per chunk ca
    # (cells are (ca, cb) sorted so a-side chunks are contiguous); for the "b"
    # side one gather per cell.
    def set_gathers(cellB):
        a_g, b_g = [], []
        off = 0
        for ca in range(NCHUNK):
            row = cellB[ca * NCHUNK:(ca + 1) * NCHUNK]
            n_slots = int(row.sum())
            if n_slots:
                a_g.append(dict(c=ca, dst_slot=off, n_idx=n_slots * P))
            o2 = off
            for cb in range(NCHUNK):
                if cellB[ca * NCHUNK + cb]:
                    b_g.append(dict(c=cb, dst_slot=o2,
                                    n_idx=int(cellB[ca * NCHUNK + cb]) * P))
                    o2 += int(cellB[ca * NCHUNK + cb])
            off += n_slots
        return a_g, b_g

    plan.gA = set_gathers(plan.setA[0])
    plan.gB = set_gathers(plan.setB[0])
    plan.gC = set_gathers(plan.setC[0])

    # ---- build per-core input arrays ----
    x0 = np.concatenate([np.asarray(user_emb, np.float32),
                         np.asarray(item_emb, np.float32)])
    x0_p = np.zeros((NT, EMB), np.float32)
    x0_p[perm] = x0
    iota = np.tile(np.arange(P, dtype=np.float32)[None, :], (P, 1)).astype(NPDT)

    in_maps = []
    for i in range(NCORES):
        idx_blocks = []
        srows = np.zeros((P, plan.tot_slots), np.float32)
        svals = np.zeros((P, plan.tot_slots), np.float32)
        for g in plan.gathers:
            tl = sb_tiles[g["sb"]]
            c = g["c"]
            parts = []
            for t in tl:
                bslots = int(B[t, c])
                if bslots == 0:
                    continue
                cellk = (i * TILES + t) * NCHUNK + c
                st, en = starts[cellk], starts[cellk + 1]
                n = int(en - st)
                want = bslots * P
                lc = lcol_s[st:en]
                lr = lrow_s[st:en]
                vv = vals_s[st:en]
                pad = want - n
                lc = np.concatenate([lc, np.zeros(pad, np.int16)])
                lr = np.concatenate([lr, np.zeros(pad, np.float32)])
                vv = np.concatenate([vv, np.zeros(pad, np.float32)])
                parts.append((lc, lr, vv, t))
                # stream layout: slot k of (t,c) region -> global slot
                so = slot_off[t, c]
                srows[:, so:so + bslots] = lr.reshape(bslots, P).T
                svals[:, so:so + bslots] = vv.reshape(bslots, P).T
            lc_all = np.concatenate([p[0] for p in parts]) if parts else \
                np.zeros(0, np.int16)
            assert lc_all.size == g["n_idx"]
            idx_blocks.append(_wrap_idx(lc_all))
        # batch idx blocks
        def batch_idx(plan_set, gset):
            (a, b, m) = plan_set[1][i]
            out = []
            for g in gset[0]:
                sl = a[g["dst_slot"] * P: g["dst_slot"] * P + g["n_idx"]]
                out.append(_wrap_idx((sl - g["c"] * CHUNK).astype(np.int16)))
            for g in gset[1]:
                sl = b[g["dst_slot"] * P: g["dst_slot"] * P + g["n_idx"]]
                out.append(_wrap_idx((sl - g["c"] * CHUNK).astype(np.int16)))
            return out, m

        bA, mA = batch_idx(plan.setA, plan.gA)
        bB, mB = batch_idx(plan.setB, plan.gB)
        bC, mC = batch_idx(plan.setC, plan.gC)
        idx_blocks += bA + bB + bC
        idx_all = np.concatenate(idx_blocks, axis=1)   # [16, cols]
        masks = np.stack([  # [P, sA+sB+sC]
            np.concatenate([mA.reshape(-1, P).T, mB.reshape(-1, P).T,
                            mC.reshape(-1, P).T], axis=1)
        ])[0].astype(np.float32)

        own0 = x0_p[i * RPC:(i + 1) * RPC]

        in_maps.append({
            "own0": own0.astype(NPDT),
            "idxs": np.ascontiguousarray(idx_all),
            "srows": srows.astype(NPDT),
            "svals": svals.astype(NPDT),
            "iota_in": iota,
            "lmask": masks,
        })

    plan.idx_cols_total = in_maps[0]["idxs"].shape[1]
    # record batch gather idx column offsets (shared across cores)
    col = plan.idx_cols_edges
    plan.batch_cols = []
    for gset in (plan.gA, plan.gB, plan.gC):
        for g in gset[0] + gset[1]:
            plan.batch_cols.append(col)
            col += g["n_idx"] // 16
    return plan, in_maps


# ============================================================
# Bass program
# ============================================================

def build_nc(plan):
    nc = bacc.Bacc("TRN2", target_bir_lowering=False, debug=False,
                   num_devices=1 if DEBUG_SINGLE else NCORES)
    f32 = mybir.dt.float32

    own0 = nc.dram_tensor("own0", [RPC, EMB], DT, kind="ExternalInput")
    idxs = nc.dram_tensor("idxs", [16, plan.idx_cols_total], mybir.dt.int16,
                          kind="ExternalInput")
    srows_in = nc.dram_tensor("srows", [P, plan.tot_slots], DT,
                              kind="ExternalInput")
    svals_in = nc.dram_tensor("svals", [P, plan.tot_slots], DT,
                              kind="ExternalInput")
    stage0 = nc.dram_tensor("stage0", [RPC, EMB], DT)
    table0 = nc.dram_tensor("table0s", [NT, EMB], DT, addr_space="Shared")
    iota_in = nc.dram_tensor("iota_in", [P, P], DT, kind="ExternalInput")
    lmask_in = nc.dram_tensor("lmask", [P, plan.sA + plan.sB + plan.sC], f32,
                              kind="ExternalInput")
    partials = nc.dram_tensor("partials", [1, 8], f32, kind="ExternalOutput")

    acc_d = nc.dram_tensor("acc_d", [RPC, EMB], f32)
    stage = nc.dram_tensor("stage", [RPC, EMB], DT)
    fstage = nc.dram_tensor("fstage", [RPC, EMB], f32)
    tables = [table0,
              nc.dram_tensor("table1", [NT, EMB], DT, addr_space="Shared"),
              nc.dram_tensor("table2", [NT, EMB], DT, addr_space="Shared")]
    finalT = nc.dram_tensor("finalT", [NT, EMB], f32, addr_space="Shared")

    groups = [list(range(NCORES))]

    with tile.TileContext(nc, num_cores=NCORES) as tc:
        with (
            tc.tile_pool(name="persist", bufs=1) as pers,
            tc.tile_pool(name="spool", bufs=6) as spool,
            tc.tile_pool(name="xpool", bufs=6) as xpool,
            tc.tile_pool(name="psum", bufs=4, space="PSUM") as pp,
        ):
            # ---- persistent loads ----
            idx_t = pers.tile([128, plan.idx_cols_total], mybir.dt.int16)
            for kk in range(8):
                nc.sync.dma_start(out=idx_t[16 * kk:16 * (kk + 1), :],
                                  in_=idxs[:, :])
            srow_b = pers.tile([P, plan.tot_slots], DT, tag="srow_b")
            nc.sync.dma_start(out=srow_b[:], in_=srows_in[:, :])
            srow_t = pers.tile([P, plan.tot_slots], mybir.dt.float32)
            nc.vector.tensor_copy(out=srow_t[:], in_=srow_b[:])
            sval_b = pers.tile([P, plan.tot_slots], DT, tag="sval_b")
            nc.sync.dma_start(out=sval_b[:], in_=svals_in[:, :])
            sval_t = pers.tile([P, plan.tot_slots], mybir.dt.float32)
            nc.vector.tensor_copy(out=sval_t[:], in_=sval_b[:])
            # build replicated table0 from own rows
            nc.sync.dma_start(out=stage0[:, :], in_=own0[:, :])
            if DEBUG_SINGLE or DEBUG_LOCAL_COLL:
                nc.sync.dma_start(out=table0[0:RPC, :], in_=stage0[:, :])
            else:
                nc.gpsimd.collective_compute(
                    "AllGather", mybir.AluOpType.bypass,
                    replica_groups=groups,
                    ins=[stage0[:, :]], outs=[table0[:, :]],
                )
            iota_t = pers.tile([P, P], DT)
            nc.sync.dma_start(out=iota_t[:], in_=iota_in[:, :])
            mask_t = pers.tile([P, plan.sA + plan.sB + plan.sC],
                               mybir.dt.float32)
            nc.sync.dma_start(out=mask_t[:], in_=lmask_in[:, :])
            ones_t = pers.tile([P, 1], mybir.dt.float32)
            nc.gpsimd.memset(ones_t[:], 1.0)

            # ---- 3 SpMM layers ----
            gpool_cm = tc.tile_pool(name="gpool", bufs=2)
            gpool = gpool_cm.__enter__()
            for layer in range(DEBUG_LAYERS):
                tbl = tables[layer]
                sb_list = plan.sb_tiles if DEBUG_SB_LIMIT is None \
                    else plan.sb_tiles[:DEBUG_SB_LIMIT]
                for s, tl in enumerate(sb_list):
                    g_t = gpool.tile([P, plan.g_slots, EMB], DT, tag="G")
                    sb_base = plan.sb_slot_base[s]
                    for g in plan.gathers:
                        if g["sb"] != s:
                            continue
                        c = g["c"]
                        crows = min(CHUNK, NT - c * CHUNK)
                        for off in range(0, g["n_idx"], GMAX):
                            n = min(GMAX, g["n_idx"] - off)
                            nc.gpsimd.dma_gather(
                                out_ap=g_t[:, g["dst_slot"] + off // P:
                                           g["dst_slot"] + (off + n) // P, :],
                                in_ap=tbl[c * CHUNK: c * CHUNK + crows, :],
                                idxs_ap=idx_t[:, g["idx_col"] + off // 16:
                                              g["idx_col"] + (off + n) // 16],
                                num_idxs=n,
                                num_idxs_reg=n,
                                elem_size=EMB,
                            )
                    for t in tl:
                        nslots = int(plan.B[t].sum())
                        ps = pp.tile([P, EMB], mybir.dt.float32, tag="ps",
                                     space="PSUM")
                        k = 0
                        for c in range(NCHUNK):
                            for j in range(int(plan.B[t, c])):
                                gs = plan.slot_off[t, c] + j   # global slot
                                s_t = spool.tile([P, P], DT, tag="S")
                                nc.vector.tensor_scalar(
                                    out=s_t[:],
                                    in0=iota_t[:],
                                    scalar1=srow_t[:, gs, None],
                                    scalar2=sval_t[:, gs, None],
                                    op0=mybir.AluOpType.is_equal,
                                    op1=mybir.AluOpType.mult,
                                )
                                nc.tensor.matmul(
                                    out=ps[:],
                                    lhsT=s_t[:],
                                    rhs=g_t[:, gs - sb_base, :],
                                    start=(k == 0),
                                    stop=(k == nslots - 1),
                                )
                                k += 1
                        # evacuate + accumulate
                        x_t = xpool.tile([P, EMB], mybir.dt.float32, tag="X")
                        if nslots:
                            nc.scalar.activation(
                                x_t[:], ps[:], mybir.ActivationFunctionType.Copy)
                        else:
                            nc.vector.memset(x_t[:], 0.0)
                        if layer == 0:
                            prev_b = xpool.tile([P, EMB], DT, tag="PREVB")
                            nc.sync.dma_start(out=prev_b[:],
                                              in_=own0[t * P:(t + 1) * P, :])
                            prev_t = xpool.tile([P, EMB], mybir.dt.float32,
                                                tag="PREV")
                            nc.scalar.activation(
                                prev_t[:], prev_b[:],
                                mybir.ActivationFunctionType.Copy)
                        else:
                            prev_t = xpool.tile([P, EMB], mybir.dt.float32,
                                                tag="PREV")
                            nc.sync.dma_start(
                                out=prev_t[:],
                                in_=acc_d[t * P:(t + 1) * P, :])
                        na = xpool.tile([P, EMB], mybir.dt.float32, tag="NA")
                        nc.vector.tensor_tensor(out=na[:], in0=x_t[:],
                                                in1=prev_t[:],
                                                op=mybir.AluOpType.add)
                        if layer < N_LAYERS - 1:
                            nc.sync.dma_start(out=acc_d[t * P:(t + 1) * P, :],
                                              in_=na[:])
                            st_t = xpool.tile([P, EMB], DT, tag="ST")
                            nc.scalar.activation(
                                st_t[:], x_t[:],
                                mybir.ActivationFunctionType.Copy)
                            nc.sync.dma_start(out=stage[t * P:(t + 1) * P, :],
                                              in_=st_t[:])
                        else:
                            fin_t = xpool.tile([P, EMB], mybir.dt.float32,
                                               tag="FIN")
                            nc.scalar.mul(fin_t[:], na[:],
                                          1.0 / (N_LAYERS + 1))
                            nc.sync.dma_start(out=fstage[t * P:(t + 1) * P, :],
                                              in_=fin_t[:])
                if DEBUG_SINGLE or DEBUG_LOCAL_COLL:
                    if layer < N_LAYERS - 1:
                        nc.sync.dma_start(out=tables[layer + 1][0:RPC, :],
                                          in_=stage[:, :])
                    else:
                        nc.sync.dma_start(out=finalT[0:RPC, :],
                                          in_=fstage[:, :])
                elif layer < N_LAYERS - 1:
                    nc.gpsimd.collective_compute(
                        "AllGather", mybir.AluOpType.bypass,
                        replica_groups=groups,
                        ins=[stage[:, :]], outs=[tables[layer + 1][:, :]],
                    )
                else:
                    nc.gpsimd.collective_compute(
                        "AllGather", mybir.AluOpType.bypass,
                        replica_groups=groups,
                        ins=[fstage[:, :]], outs=[finalT[:, :]],
                    )

            gpool_cm.__exit__(None, None, None)

            # ---- loss phase ----
            lpool_cm = tc.tile_pool(name="lpool", bufs=1)
            gpool = lpool_cm.__enter__()
            f32t = mybir.dt.float32
            if DEBUG_SKIP_LOSS:
                zz = pers.tile([1, 8], f32t)
                nc.vector.memset(zz[:], 0.0)
                nc.sync.dma_start(out=partials[:, :], in_=zz[:])
            else:
                part_t = pers.tile([P, 8], f32t)
                nc.vector.memset(part_t[:], 0.0)

                bcol = iter(plan.batch_cols)

                def gather_set(gset, nslots):
                    a_t = gpool.tile([P, max(nslots, 1), EMB], f32t, tag="BA")
                    b_t = gpool.tile([P, max(nslots, 1), EMB], f32t, tag="BB")
                    for dst, glist in ((a_t, gset[0]), (b_t, gset[1])):
                        for g in glist:
                            col = next(bcol)
                            c = g["c"]
                            crows = min(CHUNK, NT - c * CHUNK)
                            for off in range(0, g["n_idx"], GMAX):
                                n = min(GMAX, g["n_idx"] - off)
                                nc.gpsimd.dma_gather(
                                    out_ap=dst[:, g["dst_slot"] + off // P:
                                               g["dst_slot"] + (off + n) // P, :],
                                    in_ap=finalT[c * CHUNK: c * CHUNK + crows, :],
                                    idxs_ap=idx_t[:, col + off // 16:
                                                  col + (off + n) // 16],
                                    num_idxs=n,
                                    num_idxs_reg=n,
                                    elem_size=EMB,
                                )
                    return a_t, b_t

                def dots_sig(a_t, b_t, nslots):
                    prod = gpool.tile([P, nslots, EMB], f32t, tag="PR")
                    nc.vector.tensor_tensor(out=prod[:], in0=a_t[:, :nslots, :],
                                            in1=b_t[:, :nslots, :],
                                            op=mybir.AluOpType.mult)
                    d_t = spool.tile([P, nslots], f32t, tag="D")
                    nc.vector.tensor_reduce(out=d_t[:], in_=prod[:],
                                            axis=mybir.AxisListType.X,
                                            op=mybir.AluOpType.add)
                    pr_t = spool.tile([P, nslots], f32t, tag="PRS")
                    nc.scalar.activation(pr_t[:], d_t[:],
                                         mybir.ActivationFunctionType.Sigmoid)
                    return pr_t

                def masked_sum(x_t, m_ap, nslots, out_col):
                    tmp = spool.tile([P, nslots], f32t, tag="MS")
                    nc.vector.tensor_tensor(out=tmp[:], in0=x_t[:],
                                            in1=m_ap,
                                            op=mybir.AluOpType.mult)
                    nc.vector.tensor_reduce(out=part_t[:, out_col, None],
                                            in_=tmp[:],
                                            axis=mybir.AxisListType.X,
                                            op=mybir.AluOpType.add)

                # set A: (u, pos)
                mA = mask_t[:, :plan.sA]
                a_t, b_t = gather_set(plan.gA, plan.sA)
                predA = dots_sig(a_t, b_t, plan.sA)
                lpA = spool.tile([P, plan.sA], f32t, tag="LPA")
                nc.scalar.activation(lpA[:], predA[:],
                                     mybir.ActivationFunctionType.Ln)
                plpA = spool.tile([P, plan.sA], f32t, tag="PLPA")
                nc.vector.tensor_tensor(out=plpA[:], in0=predA[:], in1=lpA[:],
                                        op=mybir.AluOpType.mult)
                masked_sum(lpA, mA, plan.sA, 0)       # q0 = sum ln(pred_pos)
                masked_sum(predA, mA, plan.sA, 2)     # q2a = sum pred_pos
                masked_sum(plpA, mA, plan.sA, 3)      # q3a = sum pred*ln(pred)

                # set B: (u, neg)
                mB = mask_t[:, plan.sA:plan.sA + plan.sB]
                a_t, b_t = gather_set(plan.gB, plan.sB)
                predB = dots_sig(a_t, b_t, plan.sB)
                l1mB = spool.tile([P, plan.sB], f32t, tag="L1MB")
                nc.scalar.activation(l1mB[:], predB[:],
                                     mybir.ActivationFunctionType.Ln,
                                     bias=1.0, scale=-1.0)
                lpB = spool.tile([P, plan.sB], f32t, tag="LPB")
                nc.scalar.activation(lpB[:], predB[:],
                                     mybir.ActivationFunctionType.Ln)
                plpB = spool.tile([P, plan.sB], f32t, tag="PLPB")
                nc.vector.tensor_tensor(out=plpB[:], in0=predB[:], in1=lpB[:],
                                        op=mybir.AluOpType.mult)
                masked_sum(l1mB, mB, plan.sB, 1)      # q1 = sum ln(1-pred_neg)
                masked_sum(predB, mB, plan.sB, 4)     # q2b = sum pred_neg
                masked_sum(plpB, mB, plan.sB, 5)      # q3b

                # set C: (su, si)
                mC = mask_t[:, plan.sA + plan.sB:]
                a_t, b_t = gather_set(plan.gC, plan.sC)
                predC = dots_sig(a_t, b_t, plan.sC)
                masked_sum(predC, mC, plan.sC, 6)     # q4 = sum pred_ul

                # cross-partition sum via matmul with ones
                pps = pp.tile([1, 8], f32t, tag="pps", space="PSUM")
                nc.tensor.matmul(out=pps[:], lhsT=ones_t[:], rhs=part_t[:],
                                 start=True, stop=True)
                res_t = pers.tile([1, 8], f32t)
                nc.scalar.activation(res_t[:], pps[:],
                                     mybir.ActivationFunctionType.Copy)
                nc.sync.dma_start(out=partials[:, :], in_=res_t[:])
            lpool_cm.__exit__(None, None, None)

    nc.compile()
    return nc


# ============================================================
# Public entry
# ============================================================

def host_combine(results):
    q = np.zeros(8, np.float64)
    for r in results:
        q += r["partials"].reshape(-1).astype(np.float64)
    B2 = 2.0 * BATCH
    bce = -(q[0] + q[1]) / B2
    pred_avg = (q[2] + q[4]) / B2
    pred_ul_avg = q[6] / B2
    gamma_term = (q[3] + q[5]) / B2
    info = ALPHA * (-pred_avg * np.log(pred_ul_avg)
                    - (1.0 - pred_avg) * np.log(1.0 - pred_ul_avg)) \
        + GAMMA * gamma_term
    return np.float32(bce), np.float32(info)


def kernel(**inputs):
    plan, in_maps = host_pack(**inputs)
    nc = build_nc(plan)
    res = run_bass_kernel_spmd(nc, in_maps, core_ids=list(range(NCORES)))
    return host_combine(res.results)


if __name__ == "__main__":
    pass

